# revision 82
# baseline (speedup 1.0000x reference)
"""Trainium2 Bass kernel for GQA MultiHeadAttention (B=1, S=2048, D=4096,
H=32 query heads, HKV=8 kv heads, DK=DV=128) on 8 NeuronCores.

Sharding: core c owns query heads 4c..4c+3 and kv head c for the projections
and attention (tensor-parallel over heads); the output projection is
sequence-sharded: a per-head AllToAll redistributes the attention output so
core c holds all 4096 attention features for its 256 sequence columns, then
each core computes out[:, own 256 cols] against the full (permuted) Wd.

Phase layout per core:
  P1 interleaved projections: per 8-chunk block of the D contraction,
     K, V, Qh0..3 round-robin over two 2-bank PSUM pools, partial sums
     accumulated in SBUF; kT/vT/qT chunks stream interleaved so the PE
     never waits on any single tensor's DMA.
  P2 attention: head pairs interleaved with a one-qb stagger so one
     head's softmax tail always overlaps the other's matmul-dense middle;
     scores in double-buffered 2-k-tile groups -> one batched exp each;
     causal mask on DVE (second diagonal group width-restricted); PV
     accumulated in PSUM across the row; softmax denominator accumulated
     on DVE + one ones-matmul. Per-head AllToAll fires as soon as that
     head finishes.
  P3 output projection, one pass per head-group, od-quarters so wd
     sub-chunks stay resident across both q-halves; accumulates into an
     SBUF fp32 buffer reusing the Q-accumulator slot; full Wd streamed
     through the same pool as the qT stream.

Self-contained: hardcodes all shapes; inputs are the full unsharded tensors
keyed as in the problem's setup_inputs().
"""

import numpy as np
import ml_dtypes

import concourse.bacc as bacc
import concourse.mybir as mybir
from concourse.tile import TileContext
from concourse.bass_utils import run_bass_kernel_spmd

BF16 = mybir.dt.bfloat16
F32 = mybir.dt.float32

N_CORES = 8
S = 2048            # sequence length
D = 4096            # model dim
DK = 128            # head dim
NH_LOC = 4          # query heads per core
FLOC = NH_LOC * DK  # per-core attention features (512)
NDC = D // 128      # contraction chunks of 128 over D (32)
SB = 512            # q/s block width
NSB = S // SB       # 4
NST = S // 128      # 16 seq tiles of 128
SLOC = S // N_CORES  # per-core output seq columns (256)
NBLK = 4            # projection blocks
BLK = NDC // NBLK   # 8 dc per block
OQ = 1024           # output-projection od quarter width

_DMA_TYPES = ("InstDMACopy", "InstDMATranspose")


def _legalize_dma_waits(nc):
    """DMA pseudo-instructions encode at most ONE sem wait (the ISA events
    slot). If Tile's sem assignment leaves more on a DMA, walrus rejects it
    ("Too many sync wait commands"). Hoist all but the last wait onto fresh
    nop instructions inserted immediately before the DMA on the same engine —
    the sequencer executes them in order, so semantics are identical."""
    ctr = 0
    for f in nc.m.functions:
        for blk in f.blocks:
            out = []
            changed = False
            for inst in blk.instructions:
                si = inst.sync_info
                if (
                    si is not None
                    and len(si.on_wait) > 1
                    and type(inst).__name__ in _DMA_TYPES
                ):
                    waits = list(si.on_wait)
                    for w in waits[:-1]:
                        nop = mybir.InstNoOp(
                            name=f"I-dmawaitfix-{ctr}", ins=[], outs=[]
                        )
                        ctr += 1
                        nop.engine = inst.engine
                        nop.sync_info = mybir.SyncInfo(on_wait=[w], on_update=[])
                        out.append(nop)
                    inst.sync_info = mybir.SyncInfo(
                        on_wait=[waits[-1]], on_update=list(si.on_update)
                    )
                    changed = True
                out.append(inst)
            if changed:
                blk.instructions = out
    return ctr


def _build(nrep=1):
    nc = bacc.Bacc("TRN2", target_bir_lowering=False, num_devices=N_CORES,
                   dynamic_dma_scratch_size=2048)

    # ---- I/O (host pre-layouts everything partition-major) ----
    qr = nc.dram_tensor("qr", [128, NDC, S], BF16, kind="ExternalInput")
    kr = nc.dram_tensor("kr", [128, NDC, S], BF16, kind="ExternalInput")
    vr = nc.dram_tensor("vr", [128, NDC, S], BF16, kind="ExternalInput")
    wq = nc.dram_tensor("wq", [128, NDC, FLOC], BF16, kind="ExternalInput")
    wk = nc.dram_tensor("wk", [128, NDC, DK], BF16, kind="ExternalInput")
    wv = nc.dram_tensor("wv", [128, NDC, DK], BF16, kind="ExternalInput")
    wd = nc.dram_tensor("wd", [128, NDC, D], BF16, kind="ExternalInput")
    masks = nc.dram_tensor("masks", [128, 4, SB], BF16, kind="ExternalInput")
    ident = nc.dram_tensor("ident", [128, 128], BF16, kind="ExternalInput")
    outS = nc.dram_tensor("outS", [SLOC, D], F32, kind="ExternalOutput")

    with TileContext(nc) as tc:
        with (
            tc.tile_pool(name="consts", bufs=1) as consts,
            tc.tile_pool(name="wqp", bufs=2) as wqp,
            tc.tile_pool(name="wkvp", bufs=2) as wkvp,
            tc.tile_pool(name="streamA", bufs=16) as streamA,
            tc.tile_pool(name="ktp", bufs=5) as ktp,
            tc.tile_pool(name="vtp", bufs=5) as vtp,
            tc.tile_pool(name="bigacc", bufs=1) as bigacc,
            tc.tile_pool(name="finals", bufs=1) as finals,
            tc.tile_pool(name="epool", bufs=3) as epool,
            tc.tile_pool(name="eaccp", bufs=2) as eaccp,
            tc.tile_pool(name="recp", bufs=2) as recp,
            tc.tile_pool(name="atout", bufs=2) as atout,
            tc.tile_pool(name="featp", bufs=2) as featp,
            tc.tile_pool(name="psA", bufs=2, space="PSUM") as psA,
            tc.tile_pool(name="psB", bufs=2, space="PSUM") as psB,
            tc.tile_pool(name="dram", bufs=1, space="DRAM") as dram,
        ):
            def one_rep(rep):
                ones_sb = consts.tile([128, 128], BF16, name="ones_sb")
                nc.vector.memset(ones_sb[:], 1.0)
                # PE warmup: dummy matmuls ramp the PE p-state while the
                # first stream DMAs are in flight
                warm = psB.tile([128, 2, SB], F32, name="warm", tag="psB")
                for w in range(24):
                    nc.tensor.matmul(warm[:, w % 2, 0:128], lhsT=ones_sb[:],
                                     rhs=ones_sb[:], start=(w < 2),
                                     stop=(w >= 22))

                # persistent activations
                QT_sb = finals.tile([128, NH_LOC, S], BF16, name="QT_sb")
                KT_sb = finals.tile([128, S], BF16, name="KT_sb")
                VT_tile = featp.tile([128, N_CORES, SLOC], BF16,
                                     name="VT_sb", tag="feats")
                VT_sb = VT_tile[:].rearrange("p a b -> p (a b)")
                V_sb = finals.tile([128, NST, DK], BF16, name="V_sb")

                # a2a bounce buffers (one per local head)
                ain = [dram.tile([N_CORES, 128, SLOC], BF16,
                                 name=f"ain{h}", tag=f"ain{h}")
                       for h in range(NH_LOC)]
                aout = [dram.tile([N_CORES, 128, SLOC], BF16,
                                  name=f"aout{h}", tag=f"aout{h}")
                        for h in range(NH_LOC)]

                def drain(dst_flat, na, srcs, mode, eng=None):
                    # dst_flat: AP [128, na*SB*len(srcs)]; srcs: psum tiles
                    # [128, na, SB]; mode: "copy" | "add"
                    eng = eng or nc.vector
                    for t, src in enumerate(srcs):
                        dst = dst_flat[:, t * na * SB:(t + 1) * na * SB] \
                            .rearrange("p (a b) -> p a b", a=na)
                        if mode == "copy":
                            eng.tensor_copy(dst, src[:])
                        else:
                            eng.tensor_tensor(dst, dst, src[:],
                                              mybir.AluOpType.add)

                # ---- P1: interleaved projections ----
                for blk in range(NBLK):
                    dc0 = blk * BLK
                    # K weights + K stream first (first consumers), then V, Q
                    wkc = wkvp.tile([128, BLK, DK], BF16, name="wkc", tag="wkc")
                    nc.sync.dma_start(wkc[:], wk[:, dc0:dc0 + BLK, :])
                    kt = []
                    vt = []
                    qt = []
                    for i in range(BLK):
                        t = ktp.tile([128, S], BF16, name="kt_c", tag="kt")
                        nc.sync.dma_start(t[:], kr[:, dc0 + i, :])
                        kt.append(t)
                    wvc = wkvp.tile([128, BLK, DK], BF16, name="wvc", tag="wvc")
                    nc.sync.dma_start(wvc[:], wv[:, dc0:dc0 + BLK, :])
                    wqc = wqp.tile([128, BLK, FLOC], BF16, name="wqc", tag="wqc")
                    nc.sync.dma_start(wqc[:], wq[:, dc0:dc0 + BLK, :])
                    for i in range(BLK):
                        t = vtp.tile([128, S], BF16, name="vt_c", tag="vt")
                        nc.sync.dma_start(t[:], vr[:, dc0 + i, :])
                        vt.append(t)
                    for i in range(BLK):
                        t = streamA.tile([128, S], BF16, name="qt_c", tag="sa")
                        nc.sync.dma_start(t[:], qr[:, dc0 + i, :])
                        qt.append(t)

                    # K -> psA pair
                    kps = [psA.tile([128, 2, SB], F32, name=f"kps{t}", tag="psA")
                           for t in range(2)]
                    for i in range(BLK):
                        for sb in range(NSB):
                            nc.tensor.matmul(
                                kps[sb // 2][:, sb % 2, :],
                                lhsT=wkc[:, i, :],
                                rhs=kt[i][:, sb * SB:(sb + 1) * SB],
                                start=(i == 0), stop=(i == BLK - 1),
                            )
                    drain(KT_sb[:], 2, kps, "copy" if blk == 0 else "add")

                    # V -> psB pair, bf16 accumulate directly in VT_sb
                    vps = [psB.tile([128, 2, SB], F32, name=f"vps{t}", tag="psB")
                           for t in range(2)]
                    for i in range(BLK):
                        for sb in range(NSB):
                            nc.tensor.matmul(
                                vps[sb // 2][:, sb % 2, :],
                                lhsT=wvc[:, i, :],
                                rhs=vt[i][:, sb * SB:(sb + 1) * SB],
                                start=(i == 0), stop=(i == BLK - 1),
                            )
                    drain(VT_sb, 2, vps, "copy" if blk == 0 else "add")

                    # Q heads: h0,h2 -> psA pair; h1,h3 -> psB pair
                    for f in range(NH_LOC):
                        pool = psA if f % 2 == 0 else psB
                        tag = "psA" if f % 2 == 0 else "psB"
                        qps = [pool.tile([128, 2, SB], F32, name=f"qps{t}",
                                         tag=tag) for t in range(2)]
                        for i in range(BLK):
                            for sb in range(NSB):
                                nc.tensor.matmul(
                                    qps[sb // 2][:, sb % 2, :],
                                    lhsT=wqc[:, i, f * 128:(f + 1) * 128],
                                    rhs=qt[i][:, sb * SB:(sb + 1) * SB],
                                    start=(i == 0), stop=(i == BLK - 1),
                                )
                        drain(QT_sb[:, f, :], 2, qps,
                              "copy" if blk == 0 else "add")

                # V transposes: VT_sb [dv, s] -> V_sb [s, kt, dv]
                ident_sb = consts.tile([128, 128], BF16, name="ident_sb")
                nc.sync.dma_start(ident_sb[:], ident[:])
                masks_sb = consts.tile([128, 4, SB], BF16, name="masks_sb")
                nc.sync.dma_start(masks_sb[:], masks[:])
                for t in range(2):
                    tp = psB.tile([128, 2, SB], BF16, name="tp", tag="psB")
                    for i in range(8):
                        st = t * 8 + i
                        nc.tensor.transpose(
                            tp[:, i // 4, (i % 4) * 128:(i % 4 + 1) * 128],
                            VT_sb[:, st * 128:(st + 1) * 128], ident_sb[:])
                    nc.vector.tensor_copy(
                        V_sb[:, t * 8:(t + 1) * 8, :].rearrange(
                            "p (a b) c -> p a (b c)", a=2),
                        tp[:])

                # prefetch pass-0 wd pair-chunks while attention runs (SP
                # queue is past all P1 stream DMAs at this point)
                wdpre = {}
                for oqp in range(2):
                    for j in range(N_CORES):
                        if len(wdpre) >= 12:
                            break
                        t = streamA.tile([128, 2, OQ], BF16, name="wdq",
                                         tag="sa")
                        nc.sync.dma_start(
                            t[:], wd[:, 0 * 8 + j,
                                     2 * oqp * OQ:(2 * oqp + 2) * OQ]
                            .rearrange("p (a b) -> p a b", a=2))
                        wdpre[(0, oqp, j)] = t

                # ---- P2: attention ----
                # Two heads interleaved with a one-qb STAGGER: while one head
                # is at its shallow qb boundary (den/rec/normalize tail), the
                # other is mid-qb with deep PE work, so boundary latency never
                # idles the PE. Denominators via DVE accumulation.
                def attn_qb(h, qb):
                    nkt = 4 * qb + 4
                    ngrp = nkt // 2
                    pvden = psB.tile([128, 2, SB], F32, name=f"pvden{h}",
                                     tag="psB")
                    eacc = eaccp.tile([128, 2, SB], BF16, name=f"eacc{h}",
                                      tag=f"eacc{h % 2}")
                    order = list(range(ngrp))
                    if ngrp > 2:  # diagonal (masked) groups first
                        order = [ngrp - 2, ngrp - 1] + list(range(ngrp - 2))
                    for pos, g in enumerate(order):
                        first, last = pos == 0, pos == ngrp - 1
                        # second diagonal group: cols < SLOC fully masked;
                        # skip them in exp/mask/PV/eacc entirely
                        rq = SLOC if (g == ngrp - 1 and ngrp > 2) else 0
                        sc = psA.tile([128, 2, SB], F32, name="sc", tag="psA")
                        for i in range(2):
                            kt_i = 2 * g + i
                            nc.tensor.matmul(
                                sc[:, i, :],
                                lhsT=KT_sb[:, kt_i * 128:(kt_i + 1) * 128],
                                rhs=QT_sb[:, h, qb * SB:(qb + 1) * SB],
                                start=True, stop=True,
                            )
                        E = epool.tile([128, 2, SB], BF16, name="E", tag="E")
                        nc.scalar.activation(
                            E[:, :, rq:SB], sc[:, :, rq:SB],
                            mybir.ActivationFunctionType.Exp)
                        if g >= ngrp - 2:  # diagonal pair -> causal mask
                            u = g - (ngrp - 2)
                            nc.vector.tensor_tensor(
                                E[:, :, rq:SB], E[:, :, rq:SB],
                                masks_sb[:, 2 * u:2 * u + 2, rq:SB],
                                mybir.AluOpType.mult)
                        for i in range(2):
                            nc.tensor.matmul(
                                pvden[:, 0, rq:SB],
                                lhsT=V_sb[:, 2 * g + i, :],
                                rhs=E[:, i, rq:SB],
                                start=(first and i == 0),
                                stop=(last and i == 1),
                            )
                        if first:
                            nc.vector.tensor_copy(eacc[:], E[:])
                        else:
                            nc.vector.tensor_tensor(
                                eacc[:, :, rq:SB], eacc[:, :, rq:SB],
                                E[:, :, rq:SB], mybir.AluOpType.add)
                        yield
                    # qb tail: denominator matmuls, normalize, a2a input
                    nc.tensor.matmul(pvden[:, 1, :], lhsT=ones_sb[:],
                                     rhs=eacc[:, 0, :], start=True, stop=False)
                    nc.tensor.matmul(pvden[:, 1, :], lhsT=ones_sb[:],
                                     rhs=eacc[:, 1, :], start=False, stop=True)
                    rec = recp.tile([128, SB], F32, name="rec", tag="rec")
                    nc.vector.reciprocal(rec[:], pvden[:, 1, :])
                    attn_t = atout.tile([128, SB], BF16, name="attn_t",
                                        tag="attn")
                    nc.vector.tensor_tensor(attn_t[:], pvden[:, 0, :],
                                            rec[:], mybir.AluOpType.mult)
                    nc.sync.dma_start(ain[h][2 * qb], attn_t[:, 0:SLOC])
                    nc.sync.dma_start(ain[h][2 * qb + 1], attn_t[:, SLOC:SB])
                    yield

                def fire_a2a(h):
                    nc.gpsimd.collective_compute(
                        "AllToAll",
                        mybir.AluOpType.bypass,
                        replica_groups=[list(range(N_CORES))],
                        ins=[ain[h][:]],
                        outs=[aout[h][:]],
                    )

                for hp in range(NH_LOC // 2):
                    ha, hb = 2 * hp, 2 * hp + 1
                    for s in range(NSB + 1):
                        active = []
                        if s < NSB:
                            active.append(attn_qb(ha, s))
                        if s >= 1:
                            active.append(attn_qb(hb, s - 1))
                        while active:
                            nxt = []
                            for gen in reversed(active):
                                try:
                                    next(gen)
                                    nxt.append(gen)
                                except StopIteration:
                                    pass
                            active = list(reversed(nxt))
                        if s == NSB - 1:
                            fire_a2a(ha)  # head a is done one step early
                    fire_a2a(hb)

                # ---- P3: output projection, one pass per head-group ----
                out_acc = bigacc.tile([128, 2, D], F32, name="out_acc",
                                      tag="big")
                for h in range(NH_LOC):
                    feats = featp.tile([128, N_CORES, SLOC], BF16,
                                       name="feats", tag="feats")
                    for j in range(N_CORES):
                        nc.sync.dma_start(feats[:, j, :], aout[h][j])
                    for oq in range(4):
                        oqp = oq // 2
                        if oq % 2 == 0:
                            wdqp = []
                            for j in range(N_CORES):
                                if (h, oqp, j) in wdpre:
                                    wdqp.append(wdpre[(h, oqp, j)])
                                    continue
                                t = streamA.tile([128, 2, OQ], BF16,
                                                 name="wdq", tag="sa")
                                nc.sync.dma_start(
                                    t[:], wd[:, h * 8 + j,
                                             2 * oqp * OQ:(2 * oqp + 2) * OQ]
                                    .rearrange("p (a b) -> p a b", a=2))
                                wdqp.append(t)
                        wdq = [wt[:, oq % 2, :] for wt in wdqp]
                        pool = psA if oq % 2 == 0 else psB
                        tag = "psA" if oq % 2 == 0 else "psB"
                        pq = [pool.tile([128, 2, SB], F32, name=f"ops{t}",
                                        tag=tag) for t in range(2)]
                        for qh in range(2):
                            for j in range(N_CORES):
                                for t in range(2):
                                    nc.tensor.matmul(
                                        pq[qh][:, t, :],
                                        lhsT=feats[:, j,
                                                   qh * 128:(qh + 1) * 128],
                                        rhs=wdq[j][:, t * SB:(t + 1) * SB],
                                        start=(j == 0),
                                        stop=(j == N_CORES - 1),
                                    )
                        for qh in range(2):
                            if h == NH_LOC - 1 and oq == 3:
                                # very last quarter: drain+store in halves to
                                # shorten the kernel tail
                                for t in range(2):
                                    o0 = oq * OQ + t * SB
                                    dsth = out_acc[:, qh, o0:o0 + SB]
                                    nc.vector.tensor_tensor(
                                        dsth, dsth, pq[qh][:, t, :],
                                        mybir.AluOpType.add)
                                    nc.sync.dma_start(
                                        outS[qh * 128:(qh + 1) * 128,
                                             o0:o0 + SB], dsth)
                                continue
                            dst = out_acc[:, qh, oq * OQ:(oq + 1) * OQ] \
                                .rearrange("p (b c) -> p b c", c=SB)
                            if h == 0:
                                nc.vector.tensor_copy(dst, pq[qh][:])
                            else:
                                nc.vector.tensor_tensor(dst, dst, pq[qh][:],
                                                        mybir.AluOpType.add)
                            if h == NH_LOC - 1:
                                nc.sync.dma_start(
                                    outS[qh * 128:(qh + 1) * 128,
                                         oq * OQ:(oq + 1) * OQ],
                                    out_acc[:, qh, oq * OQ:(oq + 1) * OQ])

            for rep in range(nrep):
                one_rep(rep)

    nc.compile()
    _legalize_dma_waits(nc)
    nc.codegen_inst_isa_subclasses()
    return nc


_NC_CACHE = None


def _get_nc():
    global _NC_CACHE
    if _NC_CACHE is None:
        _NC_CACHE = _build()
    return _NC_CACHE


def _pm(a, nchunk, width):
    """[nchunk*128, width] -> [128, nchunk, width] partition-major bf16."""
    bf = ml_dtypes.bfloat16
    return np.ascontiguousarray(
        a.reshape(nchunk, 128, width).transpose(1, 0, 2)).astype(bf)


def _make_in_maps(q, k, v, Wq, Wk, Wv, Wd):
    bf = ml_dtypes.bfloat16
    scale = np.float32(DK) ** -0.5
    qT = np.ascontiguousarray(q.reshape(S, D).T)   # [D, S]
    kT = np.ascontiguousarray(k.reshape(S, D).T)
    vT = np.ascontiguousarray(v.reshape(S, D).T)
    qr = _pm(qT, NDC, S)
    kr = _pm(kT, NDC, S)
    vr = _pm(vT, NDC, S)

    # permuted Wd: row block (h, j) = features of global head 4j+h
    wdT = np.ascontiguousarray(Wd.T)               # [feats, od]
    blocks = []
    for h in range(NH_LOC):
        for j in range(N_CORES):
            g = 4 * j + h
            blocks.append(wdT[g * 128:(g + 1) * 128, :])
    wd_r = _pm(np.concatenate(blocks, axis=0), NDC, D)

    kp = np.arange(128, dtype=np.int32)[:, None]
    qf = np.arange(SB, dtype=np.int32)[None, :]
    masks = np.stack(
        [(qf >= kp + 128 * d).astype(np.float32) for d in range(4)], axis=1
    ).astype(bf)  # [128, 4, SB]
    ident = np.eye(128, dtype=np.float32).astype(bf)

    in_maps = []
    for c in range(N_CORES):
        fs = slice(FLOC * c, FLOC * (c + 1))
        ks = slice(DK * c, DK * (c + 1))
        in_maps.append({
            "qr": qr,
            "kr": kr,
            "vr": vr,
            "wq": _pm(np.ascontiguousarray((Wq[fs, :] * scale).T), NDC, FLOC),
            "wk": _pm(np.ascontiguousarray(Wk[ks, :].T), NDC, DK),
            "wv": _pm(np.ascontiguousarray(Wv[ks, :].T), NDC, DK),
            "wd": wd_r,
            "masks": masks,
            "ident": ident,
        })
    return in_maps


def _assemble(results):
    return np.concatenate(
        [r["outS"] for r in results], axis=0).reshape(1, S, D)


def kernel(q, k, v, Wq, Wk, Wv, Wd, _trace=False, **_ignored):
    nc = _get_nc()
    in_maps = _make_in_maps(
        np.asarray(q, np.float32), np.asarray(k, np.float32),
        np.asarray(v, np.float32), np.asarray(Wq, np.float32),
        np.asarray(Wk, np.float32), np.asarray(Wv, np.float32),
        np.asarray(Wd, np.float32),
    )
    res = run_bass_kernel_spmd(
        nc, in_maps, core_ids=list(range(N_CORES)), trace=_trace
    )
    out = _assemble(res.results)
    if _trace:
        return out, res
    return out


# revision 83
# speedup vs baseline: 1.0024x; 1.0024x over previous
"""Trainium2 Bass kernel for GQA MultiHeadAttention (B=1, S=2048, D=4096,
H=32 query heads, HKV=8 kv heads, DK=DV=128) on 8 NeuronCores.

Sharding: core c owns query heads 4c..4c+3 and kv head c for the projections
and attention (tensor-parallel over heads); the output projection is
sequence-sharded: a per-head AllToAll redistributes the attention output so
core c holds all 4096 attention features for its 256 sequence columns, then
each core computes out[:, own 256 cols] against the full (permuted) Wd.

Phase layout per core:
  P1 interleaved projections: per 8-chunk block of the D contraction,
     K, V, Qh0..3 round-robin over two 2-bank PSUM pools, partial sums
     accumulated in SBUF; kT/vT/qT chunks stream interleaved so the PE
     never waits on any single tensor's DMA.
  P2 attention: head pairs interleaved with a one-qb stagger so one
     head's softmax tail always overlaps the other's matmul-dense middle;
     scores in double-buffered 2-k-tile groups -> one batched exp each;
     causal mask on DVE (second diagonal group width-restricted); PV
     accumulated in PSUM across the row; softmax denominator accumulated
     on DVE + one ones-matmul. Per-head AllToAll fires as soon as that
     head finishes.
  P3 output projection, one pass per head-group, od-quarters so wd
     sub-chunks stay resident across both q-halves; accumulates into an
     SBUF fp32 buffer reusing the Q-accumulator slot; full Wd streamed
     through the same pool as the qT stream.

Self-contained: hardcodes all shapes; inputs are the full unsharded tensors
keyed as in the problem's setup_inputs().
"""

import numpy as np
import ml_dtypes

import concourse.bacc as bacc
import concourse.mybir as mybir
from concourse.tile import TileContext
from concourse.bass_utils import run_bass_kernel_spmd

BF16 = mybir.dt.bfloat16
F32 = mybir.dt.float32

N_CORES = 8
S = 2048            # sequence length
D = 4096            # model dim
DK = 128            # head dim
NH_LOC = 4          # query heads per core
FLOC = NH_LOC * DK  # per-core attention features (512)
NDC = D // 128      # contraction chunks of 128 over D (32)
SB = 512            # q/s block width
NSB = S // SB       # 4
NST = S // 128      # 16 seq tiles of 128
SLOC = S // N_CORES  # per-core output seq columns (256)
NBLK = 4            # projection blocks
BLK = NDC // NBLK   # 8 dc per block
OQ = 1024           # output-projection od quarter width

_DMA_TYPES = ("InstDMACopy", "InstDMATranspose")


def _legalize_dma_waits(nc):
    """DMA pseudo-instructions encode at most ONE sem wait (the ISA events
    slot). If Tile's sem assignment leaves more on a DMA, walrus rejects it
    ("Too many sync wait commands"). Hoist all but the last wait onto fresh
    nop instructions inserted immediately before the DMA on the same engine —
    the sequencer executes them in order, so semantics are identical."""
    ctr = 0
    for f in nc.m.functions:
        for blk in f.blocks:
            out = []
            changed = False
            for inst in blk.instructions:
                si = inst.sync_info
                if (
                    si is not None
                    and len(si.on_wait) > 1
                    and type(inst).__name__ in _DMA_TYPES
                ):
                    waits = list(si.on_wait)
                    for w in waits[:-1]:
                        nop = mybir.InstNoOp(
                            name=f"I-dmawaitfix-{ctr}", ins=[], outs=[]
                        )
                        ctr += 1
                        nop.engine = inst.engine
                        nop.sync_info = mybir.SyncInfo(on_wait=[w], on_update=[])
                        out.append(nop)
                    inst.sync_info = mybir.SyncInfo(
                        on_wait=[waits[-1]], on_update=list(si.on_update)
                    )
                    changed = True
                out.append(inst)
            if changed:
                blk.instructions = out
    return ctr


def _build(nrep=1):
    nc = bacc.Bacc("TRN2", target_bir_lowering=False, num_devices=N_CORES,
                   dynamic_dma_scratch_size=2048)

    # ---- I/O (host pre-layouts everything partition-major) ----
    qr = nc.dram_tensor("qr", [128, NDC, S], BF16, kind="ExternalInput")
    kr = nc.dram_tensor("kr", [128, NDC, S], BF16, kind="ExternalInput")
    vr = nc.dram_tensor("vr", [128, NDC, S], BF16, kind="ExternalInput")
    wq = nc.dram_tensor("wq", [128, NDC, FLOC], BF16, kind="ExternalInput")
    wk = nc.dram_tensor("wk", [128, NDC, DK], BF16, kind="ExternalInput")
    wv = nc.dram_tensor("wv", [128, NDC, DK], BF16, kind="ExternalInput")
    wd = nc.dram_tensor("wd", [128, NDC, D], BF16, kind="ExternalInput")
    masks = nc.dram_tensor("masks", [128, 4, SB], BF16, kind="ExternalInput")
    ident = nc.dram_tensor("ident", [128, 128], BF16, kind="ExternalInput")
    outS = nc.dram_tensor("outS", [SLOC, D], F32, kind="ExternalOutput")

    with TileContext(nc) as tc:
        with (
            tc.tile_pool(name="consts", bufs=1) as consts,
            tc.tile_pool(name="wqp", bufs=2) as wqp,
            tc.tile_pool(name="wkvp", bufs=2) as wkvp,
            tc.tile_pool(name="streamA", bufs=16) as streamA,
            tc.tile_pool(name="ktp", bufs=5) as ktp,
            tc.tile_pool(name="vtp", bufs=5) as vtp,
            tc.tile_pool(name="bigacc", bufs=1) as bigacc,
            tc.tile_pool(name="finals", bufs=1) as finals,
            tc.tile_pool(name="epool", bufs=3) as epool,
            tc.tile_pool(name="eaccp", bufs=2) as eaccp,
            tc.tile_pool(name="recp", bufs=2) as recp,
            tc.tile_pool(name="atout", bufs=2) as atout,
            tc.tile_pool(name="featp", bufs=2) as featp,
            tc.tile_pool(name="psA", bufs=2, space="PSUM") as psA,
            tc.tile_pool(name="psB", bufs=2, space="PSUM") as psB,
            tc.tile_pool(name="dram", bufs=1, space="DRAM") as dram,
        ):
            def one_rep(rep):
                ones_sb = consts.tile([128, 128], BF16, name="ones_sb")
                nc.vector.memset(ones_sb[:], 1.0)
                # PE warmup: dummy matmuls ramp the PE p-state while the
                # first stream DMAs are in flight
                warm = psB.tile([128, 2, SB], F32, name="warm", tag="psB")
                for w in range(24):
                    nc.tensor.matmul(warm[:, w % 2, 0:128], lhsT=ones_sb[:],
                                     rhs=ones_sb[:], start=(w < 2),
                                     stop=(w >= 22))

                # persistent activations
                QT_sb = finals.tile([128, NH_LOC, S], BF16, name="QT_sb")
                KT_sb = finals.tile([128, S], BF16, name="KT_sb")
                VT_tile = featp.tile([128, N_CORES, SLOC], BF16,
                                     name="VT_sb", tag="feats")
                VT_sb = VT_tile[:].rearrange("p a b -> p (a b)")
                V_sb = finals.tile([128, NST, DK], BF16, name="V_sb")

                # a2a bounce buffers (one per local head)
                ain = [dram.tile([N_CORES, 128, SLOC], BF16,
                                 name=f"ain{h}", tag=f"ain{h}")
                       for h in range(NH_LOC)]
                aout = [dram.tile([N_CORES, 128, SLOC], BF16,
                                  name=f"aout{h}", tag=f"aout{h}")
                        for h in range(NH_LOC)]

                def drain(dst_flat, na, srcs, mode, eng=None):
                    # dst_flat: AP [128, na*SB*len(srcs)]; srcs: psum tiles
                    # [128, na, SB]; mode: "copy" | "add"
                    eng = eng or nc.vector
                    for t, src in enumerate(srcs):
                        dst = dst_flat[:, t * na * SB:(t + 1) * na * SB] \
                            .rearrange("p (a b) -> p a b", a=na)
                        if mode == "copy":
                            eng.tensor_copy(dst, src[:])
                        else:
                            eng.tensor_tensor(dst, dst, src[:],
                                              mybir.AluOpType.add)

                # ---- P1: interleaved projections ----
                for blk in range(NBLK):
                    dc0 = blk * BLK
                    # K weights + K stream first (first consumers), then V, Q
                    wkc = wkvp.tile([128, BLK, DK], BF16, name="wkc", tag="wkc")
                    nc.sync.dma_start(wkc[:], wk[:, dc0:dc0 + BLK, :])
                    kt = []
                    vt = []
                    qt = []
                    for i in range(BLK):
                        t = ktp.tile([128, S], BF16, name="kt_c", tag="kt")
                        nc.sync.dma_start(t[:], kr[:, dc0 + i, :])
                        kt.append(t)
                    wvc = wkvp.tile([128, BLK, DK], BF16, name="wvc", tag="wvc")
                    nc.sync.dma_start(wvc[:], wv[:, dc0:dc0 + BLK, :])
                    wqc = wqp.tile([128, BLK, FLOC], BF16, name="wqc", tag="wqc")
                    nc.sync.dma_start(wqc[:], wq[:, dc0:dc0 + BLK, :])
                    for i in range(BLK):
                        t = vtp.tile([128, S], BF16, name="vt_c", tag="vt")
                        nc.sync.dma_start(t[:], vr[:, dc0 + i, :])
                        vt.append(t)
                    for i in range(BLK):
                        t = streamA.tile([128, S], BF16, name="qt_c", tag="sa")
                        nc.sync.dma_start(t[:], qr[:, dc0 + i, :])
                        qt.append(t)

                    # K -> psA pair
                    kps = [psA.tile([128, 2, SB], F32, name=f"kps{t}", tag="psA")
                           for t in range(2)]
                    for i in range(BLK):
                        for sb in range(NSB):
                            nc.tensor.matmul(
                                kps[sb // 2][:, sb % 2, :],
                                lhsT=wkc[:, i, :],
                                rhs=kt[i][:, sb * SB:(sb + 1) * SB],
                                start=(i == 0), stop=(i == BLK - 1),
                            )
                    drain(KT_sb[:], 2, kps, "copy" if blk == 0 else "add")

                    # V -> psB pair, bf16 accumulate directly in VT_sb
                    vps = [psB.tile([128, 2, SB], F32, name=f"vps{t}", tag="psB")
                           for t in range(2)]
                    for i in range(BLK):
                        for sb in range(NSB):
                            nc.tensor.matmul(
                                vps[sb // 2][:, sb % 2, :],
                                lhsT=wvc[:, i, :],
                                rhs=vt[i][:, sb * SB:(sb + 1) * SB],
                                start=(i == 0), stop=(i == BLK - 1),
                            )
                    drain(VT_sb, 2, vps, "copy" if blk == 0 else "add")

                    # Q heads: h0,h2 -> psA pair; h1,h3 -> psB pair
                    for f in range(NH_LOC):
                        pool = psA if f % 2 == 0 else psB
                        tag = "psA" if f % 2 == 0 else "psB"
                        qps = [pool.tile([128, 2, SB], F32, name=f"qps{t}",
                                         tag=tag) for t in range(2)]
                        for i in range(BLK):
                            for sb in range(NSB):
                                nc.tensor.matmul(
                                    qps[sb // 2][:, sb % 2, :],
                                    lhsT=wqc[:, i, f * 128:(f + 1) * 128],
                                    rhs=qt[i][:, sb * SB:(sb + 1) * SB],
                                    start=(i == 0), stop=(i == BLK - 1),
                                )
                        drain(QT_sb[:, f, :], 2, qps,
                              "copy" if blk == 0 else "add")

                # V transposes: VT_sb [dv, s] -> V_sb [s, kt, dv]
                ident_sb = consts.tile([128, 128], BF16, name="ident_sb")
                nc.sync.dma_start(ident_sb[:], ident[:])
                masks_sb = consts.tile([128, 4, SB], BF16, name="masks_sb")
                nc.sync.dma_start(masks_sb[:], masks[:])
                for t in range(2):
                    tp = psB.tile([128, 2, SB], BF16, name="tp", tag="psB")
                    for i in range(8):
                        st = t * 8 + i
                        nc.tensor.transpose(
                            tp[:, i // 4, (i % 4) * 128:(i % 4 + 1) * 128],
                            VT_sb[:, st * 128:(st + 1) * 128], ident_sb[:])
                    nc.vector.tensor_copy(
                        V_sb[:, t * 8:(t + 1) * 8, :].rearrange(
                            "p (a b) c -> p a (b c)", a=2),
                        tp[:])

                # prefetch pass-0 wd pair-chunks while attention runs (SP
                # queue is past all P1 stream DMAs at this point)
                wdpre = {}
                for oqp in range(2):
                    for j in range(N_CORES):
                        if len(wdpre) >= 12:
                            break
                        t = streamA.tile([128, 2, OQ], BF16, name="wdq",
                                         tag="sa")
                        nc.sync.dma_start(
                            t[:], wd[:, 0 * 8 + j,
                                     2 * oqp * OQ:(2 * oqp + 2) * OQ]
                            .rearrange("p (a b) -> p a b", a=2))
                        wdpre[(0, oqp, j)] = t

                # ---- P2: attention ----
                # Two heads interleaved with a one-qb STAGGER: while one head
                # is at its shallow qb boundary (den/rec/normalize tail), the
                # other is mid-qb with deep PE work, so boundary latency never
                # idles the PE. Denominators via DVE accumulation.
                def attn_qb(h, qb):
                    nkt = 4 * qb + 4
                    ngrp = nkt // 2
                    pvden = psB.tile([128, 2, SB], F32, name=f"pvden{h}",
                                     tag="psB")
                    eacc = eaccp.tile([128, 2, SB], BF16, name=f"eacc{h}",
                                      tag=f"eacc{h % 2}")
                    order = list(range(ngrp))
                    if ngrp > 2:  # diagonal (masked) groups first
                        order = [ngrp - 2, ngrp - 1] + list(range(ngrp - 2))
                    for pos, g in enumerate(order):
                        first, last = pos == 0, pos == ngrp - 1
                        # second diagonal group: cols < SLOC fully masked;
                        # skip them in exp/mask/PV/eacc entirely
                        rq = SLOC if (g == ngrp - 1 and ngrp > 2) else 0
                        sc = psA.tile([128, 2, SB], F32, name="sc", tag="psA")
                        for i in range(2):
                            kt_i = 2 * g + i
                            nc.tensor.matmul(
                                sc[:, i, :],
                                lhsT=KT_sb[:, kt_i * 128:(kt_i + 1) * 128],
                                rhs=QT_sb[:, h, qb * SB:(qb + 1) * SB],
                                start=True, stop=True,
                            )
                        E = epool.tile([128, 2, SB], BF16, name="E", tag="E")
                        nc.scalar.activation(
                            E[:, :, rq:SB], sc[:, :, rq:SB],
                            mybir.ActivationFunctionType.Exp)
                        if g >= ngrp - 2:  # diagonal pair -> causal mask
                            u = g - (ngrp - 2)
                            nc.vector.tensor_tensor(
                                E[:, :, rq:SB], E[:, :, rq:SB],
                                masks_sb[:, 2 * u:2 * u + 2, rq:SB],
                                mybir.AluOpType.mult)
                        for i in range(2):
                            nc.tensor.matmul(
                                pvden[:, 0, rq:SB],
                                lhsT=V_sb[:, 2 * g + i, :],
                                rhs=E[:, i, rq:SB],
                                start=(first and i == 0),
                                stop=(last and i == 1),
                            )
                        if first:
                            nc.vector.tensor_copy(eacc[:], E[:])
                        else:
                            # masked-out cols of the final diagonal group are
                            # exact zeros in E; skip them in the accumulation
                            # even when exp/PV ran full width (qb0)
                            ra = SLOC if g == ngrp - 1 else rq
                            nc.vector.tensor_tensor(
                                eacc[:, :, ra:SB], eacc[:, :, ra:SB],
                                E[:, :, ra:SB], mybir.AluOpType.add)
                        yield
                    # qb tail: denominator matmuls, normalize, a2a input
                    nc.tensor.matmul(pvden[:, 1, :], lhsT=ones_sb[:],
                                     rhs=eacc[:, 0, :], start=True, stop=False)
                    nc.tensor.matmul(pvden[:, 1, :], lhsT=ones_sb[:],
                                     rhs=eacc[:, 1, :], start=False, stop=True)
                    rec = recp.tile([128, SB], F32, name="rec", tag="rec")
                    nc.vector.reciprocal(rec[:], pvden[:, 1, :])
                    attn_t = atout.tile([128, SB], BF16, name="attn_t",
                                        tag="attn")
                    nc.vector.tensor_tensor(attn_t[:], pvden[:, 0, :],
                                            rec[:], mybir.AluOpType.mult)
                    nc.sync.dma_start(ain[h][2 * qb], attn_t[:, 0:SLOC])
                    nc.sync.dma_start(ain[h][2 * qb + 1], attn_t[:, SLOC:SB])
                    yield

                def fire_a2a(h):
                    nc.gpsimd.collective_compute(
                        "AllToAll",
                        mybir.AluOpType.bypass,
                        replica_groups=[list(range(N_CORES))],
                        ins=[ain[h][:]],
                        outs=[aout[h][:]],
                    )

                for hp in range(NH_LOC // 2):
                    ha, hb = 2 * hp, 2 * hp + 1
                    for s in range(NSB + 1):
                        active = []
                        if s < NSB:
                            active.append(attn_qb(ha, s))
                        if s >= 1:
                            active.append(attn_qb(hb, s - 1))
                        while active:
                            nxt = []
                            for gen in reversed(active):
                                try:
                                    next(gen)
                                    nxt.append(gen)
                                except StopIteration:
                                    pass
                            active = list(reversed(nxt))
                        if s == NSB - 1:
                            fire_a2a(ha)  # head a is done one step early
                    fire_a2a(hb)

                # ---- P3: output projection, one pass per head-group ----
                out_acc = bigacc.tile([128, 2, D], F32, name="out_acc",
                                      tag="big")
                for h in range(NH_LOC):
                    feats = featp.tile([128, N_CORES, SLOC], BF16,
                                       name="feats", tag="feats")
                    for j in range(N_CORES):
                        nc.sync.dma_start(feats[:, j, :], aout[h][j])
                    for oq in range(4):
                        oqp = oq // 2
                        if oq % 2 == 0:
                            wdqp = []
                            for j in range(N_CORES):
                                if (h, oqp, j) in wdpre:
                                    wdqp.append(wdpre[(h, oqp, j)])
                                    continue
                                t = streamA.tile([128, 2, OQ], BF16,
                                                 name="wdq", tag="sa")
                                nc.sync.dma_start(
                                    t[:], wd[:, h * 8 + j,
                                             2 * oqp * OQ:(2 * oqp + 2) * OQ]
                                    .rearrange("p (a b) -> p a b", a=2))
                                wdqp.append(t)
                        wdq = [wt[:, oq % 2, :] for wt in wdqp]
                        pool = psA if oq % 2 == 0 else psB
                        tag = "psA" if oq % 2 == 0 else "psB"
                        pq = [pool.tile([128, 2, SB], F32, name=f"ops{t}",
                                        tag=tag) for t in range(2)]
                        for qh in range(2):
                            for j in range(N_CORES):
                                for t in range(2):
                                    nc.tensor.matmul(
                                        pq[qh][:, t, :],
                                        lhsT=feats[:, j,
                                                   qh * 128:(qh + 1) * 128],
                                        rhs=wdq[j][:, t * SB:(t + 1) * SB],
                                        start=(j == 0),
                                        stop=(j == N_CORES - 1),
                                    )
                        for qh in range(2):
                            if h == NH_LOC - 1 and oq == 3:
                                # very last quarter: drain+store in halves to
                                # shorten the kernel tail
                                for t in range(2):
                                    o0 = oq * OQ + t * SB
                                    dsth = out_acc[:, qh, o0:o0 + SB]
                                    nc.vector.tensor_tensor(
                                        dsth, dsth, pq[qh][:, t, :],
                                        mybir.AluOpType.add)
                                    nc.sync.dma_start(
                                        outS[qh * 128:(qh + 1) * 128,
                                             o0:o0 + SB], dsth)
                                continue
                            dst = out_acc[:, qh, oq * OQ:(oq + 1) * OQ] \
                                .rearrange("p (b c) -> p b c", c=SB)
                            if h == 0:
                                nc.vector.tensor_copy(dst, pq[qh][:])
                            else:
                                nc.vector.tensor_tensor(dst, dst, pq[qh][:],
                                                        mybir.AluOpType.add)
                            if h == NH_LOC - 1:
                                nc.sync.dma_start(
                                    outS[qh * 128:(qh + 1) * 128,
                                         oq * OQ:(oq + 1) * OQ],
                                    out_acc[:, qh, oq * OQ:(oq + 1) * OQ])

            for rep in range(nrep):
                one_rep(rep)

    nc.compile()
    _legalize_dma_waits(nc)
    nc.codegen_inst_isa_subclasses()
    return nc


_NC_CACHE = None


def _get_nc():
    global _NC_CACHE
    if _NC_CACHE is None:
        _NC_CACHE = _build()
    return _NC_CACHE


def _pm(a, nchunk, width):
    """[nchunk*128, width] -> [128, nchunk, width] partition-major bf16."""
    bf = ml_dtypes.bfloat16
    return np.ascontiguousarray(
        a.reshape(nchunk, 128, width).transpose(1, 0, 2)).astype(bf)


def _make_in_maps(q, k, v, Wq, Wk, Wv, Wd):
    bf = ml_dtypes.bfloat16
    scale = np.float32(DK) ** -0.5
    qT = np.ascontiguousarray(q.reshape(S, D).T)   # [D, S]
    kT = np.ascontiguousarray(k.reshape(S, D).T)
    vT = np.ascontiguousarray(v.reshape(S, D).T)
    qr = _pm(qT, NDC, S)
    kr = _pm(kT, NDC, S)
    vr = _pm(vT, NDC, S)

    # permuted Wd: row block (h, j) = features of global head 4j+h
    wdT = np.ascontiguousarray(Wd.T)               # [feats, od]
    blocks = []
    for h in range(NH_LOC):
        for j in range(N_CORES):
            g = 4 * j + h
            blocks.append(wdT[g * 128:(g + 1) * 128, :])
    wd_r = _pm(np.concatenate(blocks, axis=0), NDC, D)

    kp = np.arange(128, dtype=np.int32)[:, None]
    qf = np.arange(SB, dtype=np.int32)[None, :]
    masks = np.stack(
        [(qf >= kp + 128 * d).astype(np.float32) for d in range(4)], axis=1
    ).astype(bf)  # [128, 4, SB]
    ident = np.eye(128, dtype=np.float32).astype(bf)

    in_maps = []
    for c in range(N_CORES):
        fs = slice(FLOC * c, FLOC * (c + 1))
        ks = slice(DK * c, DK * (c + 1))
        in_maps.append({
            "qr": qr,
            "kr": kr,
            "vr": vr,
            "wq": _pm(np.ascontiguousarray((Wq[fs, :] * scale).T), NDC, FLOC),
            "wk": _pm(np.ascontiguousarray(Wk[ks, :].T), NDC, DK),
            "wv": _pm(np.ascontiguousarray(Wv[ks, :].T), NDC, DK),
            "wd": wd_r,
            "masks": masks,
            "ident": ident,
        })
    return in_maps


def _assemble(results):
    return np.concatenate(
        [r["outS"] for r in results], axis=0).reshape(1, S, D)


def kernel(q, k, v, Wq, Wk, Wv, Wd, _trace=False, **_ignored):
    nc = _get_nc()
    in_maps = _make_in_maps(
        np.asarray(q, np.float32), np.asarray(k, np.float32),
        np.asarray(v, np.float32), np.asarray(Wq, np.float32),
        np.asarray(Wk, np.float32), np.asarray(Wv, np.float32),
        np.asarray(Wd, np.float32),
    )
    res = run_bass_kernel_spmd(
        nc, in_maps, core_ids=list(range(N_CORES)), trace=_trace
    )
    out = _assemble(res.results)
    if _trace:
        return out, res
    return out
